# revision 1
# baseline (speedup 1.0000x reference)
"""Trainium2 kernel for nn_AxialAttention_68762426409385.

Strategy: data-parallel over the fused B*T*W row axis (8 shards, one per
NeuronCore). The device runs the dominant-cost computation — the 1x1-conv
qkv projection, a (1024x512) @ (512 x N*H) matmul = 68.7 GFLOP — as a tiled
fp32 TensorEngine matmul per shard. The lightweight attention tail
(~8 GFLOP) and the global BatchNorm are finished on host in exact fp32.
"""

import numpy as np
import concourse.bass as bass
import concourse.bacc as bacc
import concourse.tile as tile
import concourse.mybir as mybir
from concourse import bass_utils

N_HEAD = 8
BN_EPS = 1e-5
B, C, H, W, T = 4, 512, 32, 32, 16
N = B * T * W            # 2048 attention rows
NCORES = 8
NS = N // NCORES         # 256 rows per core
FREE = NS * H            # 8192 columns per core
BLK = 512                # matmul free-dim tile (one fp32 PSUM bank)
NB = FREE // BLK         # 16 blocks


USE_BF16 = True


def _build_qkv_module():
    mmdt = mybir.dt.bfloat16 if USE_BF16 else mybir.dt.float32
    dma_eng = "gpsimd" if USE_BF16 else "sync"   # SWDGE casts f32->bf16 in flight
    nc = bacc.Bacc("TRN2", target_bir_lowering=False)
    xin = nc.dram_tensor("x_sh", [C, FREE], mybir.dt.float32, kind="ExternalInput")
    win = nc.dram_tensor("wT", [C, 2 * C], mybir.dt.float32, kind="ExternalInput")
    qout = nc.dram_tensor("qkv_sh", [2 * C, FREE], mybir.dt.float32,
                          kind="ExternalOutput")

    with tile.TileContext(nc) as tc:
        with tc.tile_pool(name="wp", bufs=1) as wp, \
             tc.tile_pool(name="xp", bufs=8) as xp, \
             tc.tile_pool(name="pp", bufs=4, space="PSUM") as pp, \
             tc.tile_pool(name="op", bufs=4) as op:
            dma = getattr(nc, dma_eng)
            wts = []
            for kc in range(4):
                wt = wp.tile([128, 2 * C], mmdt, tag=f"w{kc}")
                dma.dma_start(wt[:], win[kc * 128:(kc + 1) * 128, :])
                wts.append(wt)
            for b in range(NB):
                xts = []
                for kc in range(4):
                    xt = xp.tile([128, BLK], mmdt, tag="xt")
                    dma.dma_start(
                        xt[:], xin[kc * 128:(kc + 1) * 128,
                                   b * BLK:(b + 1) * BLK])
                    xts.append(xt)
                for mc in range(8):
                    ps = pp.tile([128, BLK], mybir.dt.float32, tag="ps")
                    for kc in range(4):
                        nc.tensor.matmul(
                            ps[:],
                            lhsT=wts[kc][:, mc * 128:(mc + 1) * 128],
                            rhs=xts[kc][:],
                            start=(kc == 0), stop=(kc == 3))
                    ot = op.tile([128, BLK], mybir.dt.float32, tag="ot")
                    nc.any.tensor_copy(ot[:], ps[:])
                    nc.sync.dma_start(
                        qout[mc * 128:(mc + 1) * 128, b * BLK:(b + 1) * BLK],
                        ot[:])
    nc.compile()
    return nc


def _run_qkv(x, w_qkv, trace=False):
    """x: full (B,C,H,W,T). Returns qkv (N, 2C, H) fp32, plus profile info."""
    # (B,C,H,W,T) -> (B,T,W,C,H) -> (N, C, H)
    xp_rows = np.ascontiguousarray(np.transpose(x, (0, 4, 3, 1, 2))
                                   ).reshape(N, C, H)
    wT = np.ascontiguousarray(w_qkv.T).astype(np.float32)
    in_maps = []
    for s in range(NCORES):
        xs = np.ascontiguousarray(
            xp_rows[s * NS:(s + 1) * NS].transpose(1, 0, 2)).reshape(C, FREE)
        in_maps.append({"x_sh": xs.astype(np.float32), "wT": wT})
    nc = _build_qkv_module()
    res = bass_utils.run_bass_kernel_spmd(
        nc, in_maps, core_ids=list(range(NCORES)), trace=trace)
    shards = []
    for r in res.results:
        q = np.asarray(r["qkv_sh"]).reshape(2 * C, NS, H).transpose(1, 0, 2)
        shards.append(q)
    qkv = np.concatenate(shards, axis=0)  # (N, 2C, H)
    return qkv, res


def kernel(x, w_qkv, relative, bn_gamma, bn_beta):
    x = np.asarray(x, dtype=np.float32)
    w_qkv = np.asarray(w_qkv, dtype=np.float32)
    relative = np.asarray(relative, dtype=np.float32)
    bn_gamma = np.asarray(bn_gamma, dtype=np.float32)
    bn_beta = np.asarray(bn_beta, dtype=np.float32)

    qkv, _ = _run_qkv(x, w_qkv)

    nh = N_HEAD
    hc = C // nh                       # 64
    qkv = qkv.reshape(N, nh, 2 * hc, H)
    q = qkv[:, :, : hc // 2]           # (N, 8, 32, 32)
    k = qkv[:, :, hc // 2: hc]
    v = qkv[:, :, hc:]                 # (N, 8, 64, 32)

    ar = np.arange(H)
    rel_idx = ar[:, None] - ar[None, :] + H - 1
    all_emb = relative[:, rel_idx]     # (128, 32, 32)
    q_emb = all_emb[: hc // 2]
    k_emb = all_emb[hc // 2: hc]
    v_emb = all_emb[hc:]

    qr = np.einsum('nhci,cij->nhij', q, q_emb, optimize=True)
    kr = np.einsum('nhci,cij->nhij', k, k_emb, optimize=True)
    qk = np.einsum('nhci,nhcj->nhij', q, k, optimize=True)
    logits = qk + qr + kr
    logits -= logits.max(axis=3, keepdims=True)
    e = np.exp(logits)
    sim = e / e.sum(axis=3, keepdims=True)

    sv = np.einsum('nhij,nhcj->nhci', sim, v, optimize=True)
    sve = np.einsum('nhij,cij->nhci', sim, v_emb, optimize=True)
    stacked = np.concatenate([sv, sve], axis=-1).reshape(N, 2 * C, H)

    mean = stacked.mean(axis=(0, 2), keepdims=True)
    var = stacked.var(axis=(0, 2), keepdims=True)
    normed = (stacked - mean) / np.sqrt(var + BN_EPS)
    normed = normed * bn_gamma[None, :, None] + bn_beta[None, :, None]

    out = normed.reshape(B, T, W, C, 2, H).sum(axis=4)   # (B,T,W,C,H)
    out = out.transpose(0, 3, 4, 2, 1)                   # (B,C,H,W,T)
    return np.maximum(out + x, 0.0).astype(np.float32)



# revision 3
# speedup vs baseline: 4.3386x; 4.3386x over previous
"""Trainium2 kernel for nn_AxialAttention_68762426409385.

Strategy (v3): data-parallel over the fused B*T*W row axis, one shard per
NeuronCore, with the ENTIRE computation on device:

  1. Bass/Tile kernel (TensorEngine): the dominant-cost 1x1-conv qkv
     projection, a (1024x512) @ (512 x 8192) bf16 matmul per shard
     (68.7 GFLOP total across cores).
  2. A jitted XLA stage on the same cores, chained device-resident (the
     qkv tensor never leaves HBM): relative-position scores, softmax,
     attention-weighted values, and BatchNorm whose batch statistics are
     all-reduced across the 8 cores with lax.psum (exact global stats),
     then residual + relu.

Host work is only the (B,C,H,W,T) <-> row-major layout permutes and dtype
casts. I/O between host and device is bf16 to halve transfer cost.
"""

import os
import numpy as np
import jax
import jax.numpy as jnp
from jax.sharding import Mesh, PartitionSpec, NamedSharding
from jax.experimental.shard_map import shard_map

# Persistent compilation cache: makes cold start cheap when the NEFFs have
# been compiled before on this machine.
try:
    jax.config.update("jax_compilation_cache_dir",
                      os.path.expanduser("~/.cache/jax_comp_cache"))
    jax.config.update("jax_persistent_cache_min_entry_size_bytes", -1)
    jax.config.update("jax_persistent_cache_min_compile_time_secs", 0.5)
except Exception:
    pass

import concourse.bass as bass
import concourse.bacc as bacc
import concourse.tile as tile
import concourse.mybir as mybir
from concourse.bass2jax import (_bass_exec_p, install_neuronx_cc_hook,
                                partition_id_tensor)

N_HEAD = 8
BN_EPS = 1e-5
B, C, H, W, T = 4, 512, 32, 32, 16
N = B * T * W            # 2048 attention rows
NCORES = 8
NS = N // NCORES         # 256 rows per core
FREE = NS * H            # 8192 columns per core
BLK = 512                # matmul free-dim tile (one fp32 PSUM bank)
NB = FREE // BLK         # 16 blocks

BF16 = mybir.dt.bfloat16
F32 = mybir.dt.float32
NPBF16 = jnp.bfloat16.dtype


def _build_qkv_module():
    nc = bacc.Bacc("TRN2", target_bir_lowering=False)
    xin = nc.dram_tensor("x_sh", [C, FREE], BF16, kind="ExternalInput")
    win = nc.dram_tensor("wT", [C, 2 * C], BF16, kind="ExternalInput")
    qout = nc.dram_tensor("qkv_sh", [2 * C, FREE], BF16, kind="ExternalOutput")

    with tile.TileContext(nc) as tc:
        with tc.tile_pool(name="wp", bufs=1) as wp, \
             tc.tile_pool(name="xp", bufs=8) as xp, \
             tc.tile_pool(name="pp", bufs=4, space="PSUM") as pp, \
             tc.tile_pool(name="op", bufs=4) as op:
            wts = []
            for kc in range(4):
                wt = wp.tile([128, 2 * C], BF16, tag=f"w{kc}")
                nc.sync.dma_start(wt[:], win[kc * 128:(kc + 1) * 128, :])
                wts.append(wt)
            for b in range(NB):
                xts = []
                for kc in range(4):
                    xt = xp.tile([128, BLK], BF16, tag="xt")
                    nc.sync.dma_start(
                        xt[:], xin[kc * 128:(kc + 1) * 128,
                                   b * BLK:(b + 1) * BLK])
                    xts.append(xt)
                for mc in range(8):
                    ps = pp.tile([128, BLK], F32, tag="ps")
                    for kc in range(4):
                        nc.tensor.matmul(
                            ps[:],
                            lhsT=wts[kc][:, mc * 128:(mc + 1) * 128],
                            rhs=xts[kc][:],
                            start=(kc == 0), stop=(kc == 3))
                    ot = op.tile([128, BLK], BF16, tag="ot")
                    nc.any.tensor_copy(ot[:], ps[:])
                    nc.sync.dma_start(
                        qout[mc * 128:(mc + 1) * 128, b * BLK:(b + 1) * BLK],
                        ot[:])
    nc.compile()
    return nc


# ---------------------------------------------------------------------------
# Jitted device pipeline (built once, cached).
# ---------------------------------------------------------------------------

_STATE = {}


def _get_state():
    if _STATE:
        return _STATE
    install_neuronx_cc_hook()
    nc = _build_qkv_module()
    partition_name = (nc.partition_id_tensor.name
                      if nc.partition_id_tensor else None)
    in_names, out_names, out_avals = [], [], []
    for alloc in nc.m.functions[0].allocations:
        if not isinstance(alloc, mybir.MemoryLocationSet):
            continue
        name = alloc.memorylocations[0].name
        if alloc.kind == "ExternalInput":
            if name != partition_name:
                in_names.append(name)
        elif alloc.kind == "ExternalOutput":
            out_names.append(name)
            out_avals.append(jax.core.ShapedArray(
                tuple(alloc.tensor_shape), mybir.dt.np(alloc.dtype)))
    all_in = list(in_names) + list(out_names)
    if partition_name is not None:
        all_in.append(partition_name)

    def bass_body(x_sh, wT, zeros):
        ops = [x_sh, wT, zeros]
        if partition_name is not None:
            ops.append(partition_id_tensor())
        outs = _bass_exec_p.bind(
            *ops, out_avals=tuple(out_avals), in_names=tuple(all_in),
            out_names=tuple(out_names), lowering_input_output_aliases=(),
            sim_require_finite=True, sim_require_nnan=True, nc=nc)
        return outs[0]

    def tail_body(qkv_sh, x_sh, all_emb, g1, g2, b1, b2):
        # qkv_sh [2C, FREE] bf16; x_sh [C, FREE] bf16 (both shard-local)
        qkv = qkv_sh.astype(jnp.float32).reshape(N_HEAD, 128, NS, H)
        q = qkv[:, :32]
        k = qkv[:, 32:64]
        v = qkv[:, 64:]
        q_emb = all_emb[:32]
        k_emb = all_emb[32:64]
        v_emb = all_emb[64:]
        qk = jnp.einsum('hcni,hcnj->hnij', q, k)
        qr = jnp.einsum('hcni,cij->hnij', q, q_emb)
        kr = jnp.einsum('hcni,cij->hnij', k, k_emb)
        sim = jax.nn.softmax(qk + qr + kr, axis=3)
        sv = jnp.einsum('hnij,hcnj->hcni', sim, v)       # (8,64,NS,H)
        sve = jnp.einsum('hnij,cij->hcni', sim, v_emb)
        cnt = float(N * H)
        s1 = jax.lax.psum(sv.sum(axis=(2, 3)), 'core')
        sq1 = jax.lax.psum((sv * sv).sum(axis=(2, 3)), 'core')
        s2 = jax.lax.psum(sve.sum(axis=(2, 3)), 'core')
        sq2 = jax.lax.psum((sve * sve).sum(axis=(2, 3)), 'core')
        m1 = s1 / cnt
        va1 = sq1 / cnt - m1 * m1
        m2 = s2 / cnt
        va2 = sq2 / cnt - m2 * m2
        a1 = g1 * jax.lax.rsqrt(va1 + BN_EPS)
        a2 = g2 * jax.lax.rsqrt(va2 + BN_EPS)
        shift = a1 * m1 + a2 * m2 - b1 - b2
        xr = x_sh.astype(jnp.float32).reshape(N_HEAD, 64, NS, H)
        out = (sv * a1[:, :, None, None] + sve * a2[:, :, None, None]
               - shift[:, :, None, None] + xr)
        out = jnp.maximum(out, 0.0)
        return out.reshape(C, FREE).astype(jnp.bfloat16)

    devices = jax.devices()[:NCORES]
    mesh = Mesh(np.asarray(devices), ("core",))
    PS = PartitionSpec
    shard = NamedSharding(mesh, PS("core"))
    repl = NamedSharding(mesh, PS())

    bass_fn = jax.jit(
        shard_map(bass_body, mesh=mesh,
                  in_specs=(PS("core"),) * 3, out_specs=PS("core"),
                  check_rep=False),
        donate_argnums=(2,), keep_unused=True)
    tail_fn = jax.jit(
        shard_map(tail_body, mesh=mesh,
                  in_specs=(PS("core"), PS("core"), PS(), PS(), PS(), PS(),
                            PS()),
                  out_specs=PS("core"), check_rep=False))

    _STATE.update(dict(bass_fn=bass_fn, tail_fn=tail_fn, mesh=mesh,
                       shard=shard, repl=repl,
                       out_shape=tuple(out_avals[0].shape),
                       out_dtype=out_avals[0].dtype))
    return _STATE


def _prep_host(x, w_qkv, relative, bn_gamma, bn_beta):
    """Host-side prep: permutes + dtype casts + constant tables."""
    xp_rows = np.ascontiguousarray(np.transpose(x, (0, 4, 3, 1, 2))
                                   ).reshape(N, C, H)
    # per-core (C, FREE) shards, concatenated on axis 0
    xs_cat = np.empty((NCORES * C, FREE), dtype=NPBF16)
    for s in range(NCORES):
        xs = xp_rows[s * NS:(s + 1) * NS].transpose(1, 0, 2).reshape(C, FREE)
        xs_cat[s * C:(s + 1) * C] = xs.astype(NPBF16)
    wT = np.ascontiguousarray(w_qkv.T).astype(NPBF16)
    wt_cat = np.concatenate([wT] * NCORES, axis=0)

    ar = np.arange(H)
    rel_idx = ar[:, None] - ar[None, :] + H - 1
    all_emb = np.ascontiguousarray(relative[:, rel_idx]).astype(np.float32)

    hh = np.arange(N_HEAD)[:, None]
    ch = np.arange(64)[None, :]
    g1 = bn_gamma[hh * 128 + 2 * ch].astype(np.float32)
    g2 = bn_gamma[hh * 128 + 2 * ch + 1].astype(np.float32)
    b1 = bn_beta[hh * 128 + 2 * ch].astype(np.float32)
    b2 = bn_beta[hh * 128 + 2 * ch + 1].astype(np.float32)
    return xs_cat, wt_cat, all_emb, g1, g2, b1, b2


def _run_device(xs_cat, wt_cat, all_emb, g1, g2, b1, b2):
    """Full device phase: H2D, bass qkv, XLA tail (with BN all-reduce), D2H.

    Returns out_cat (NCORES*C, FREE) bf16 numpy.
    """
    st = _get_state()
    zeros = jnp.zeros((NCORES * st["out_shape"][0],) + st["out_shape"][1:],
                      st["out_dtype"])
    x_dev = jax.device_put(xs_cat, st["shard"])
    qkv_cat = st["bass_fn"](x_dev, wt_cat, zeros)
    out_cat = st["tail_fn"](qkv_cat, x_dev, all_emb, g1, g2, b1, b2)
    return np.asarray(out_cat)


def _finish_host(out_cat, x):
    out = out_cat.astype(np.float32).reshape(NCORES, C, NS, H)
    out = out.transpose(0, 2, 1, 3).reshape(B, T, W, C, H)
    out = out.transpose(0, 3, 4, 2, 1)                    # (B,C,H,W,T)
    return np.ascontiguousarray(out)


def kernel(x, w_qkv, relative, bn_gamma, bn_beta):
    x = np.asarray(x, dtype=np.float32)
    w_qkv = np.asarray(w_qkv, dtype=np.float32)
    relative = np.asarray(relative, dtype=np.float32)
    bn_gamma = np.asarray(bn_gamma, dtype=np.float32)
    bn_beta = np.asarray(bn_beta, dtype=np.float32)

    prepped = _prep_host(x, w_qkv, relative, bn_gamma, bn_beta)
    out_cat = _run_device(*prepped)
    return _finish_host(out_cat, x)


# revision 5
# speedup vs baseline: 4.4724x; 1.0309x over previous
"""Trainium2 kernel for nn_AxialAttention_68762426409385.

Strategy (v3): data-parallel over the fused B*T*W row axis, one shard per
NeuronCore, with the ENTIRE computation on device:

  1. Bass/Tile kernel (TensorEngine): the dominant-cost 1x1-conv qkv
     projection, a (1024x512) @ (512 x 8192) bf16 matmul per shard
     (68.7 GFLOP total across cores).
  2. A jitted XLA stage on the same cores, chained device-resident (the
     qkv tensor never leaves HBM): relative-position scores, softmax,
     attention-weighted values, and BatchNorm whose batch statistics are
     all-reduced across the 8 cores with lax.psum (exact global stats),
     then residual + relu.

Host work is only the (B,C,H,W,T) <-> row-major layout permutes and dtype
casts. I/O between host and device is bf16 to halve transfer cost.
"""

import os
import numpy as np
import jax
import jax.numpy as jnp
from jax.sharding import Mesh, PartitionSpec, NamedSharding
from jax.experimental.shard_map import shard_map

# Persistent compilation cache: makes cold start cheap when the NEFFs have
# been compiled before on this machine.
try:
    jax.config.update("jax_compilation_cache_dir",
                      os.path.expanduser("~/.cache/jax_comp_cache"))
    jax.config.update("jax_persistent_cache_min_entry_size_bytes", -1)
    jax.config.update("jax_persistent_cache_min_compile_time_secs", 0.5)
except Exception:
    pass

import concourse.bass as bass
import concourse.bacc as bacc
import concourse.tile as tile
import concourse.mybir as mybir
from concourse.bass2jax import (_bass_exec_p, install_neuronx_cc_hook,
                                partition_id_tensor)

N_HEAD = 8
BN_EPS = 1e-5
B, C, H, W, T = 4, 512, 32, 32, 16
N = B * T * W            # 2048 attention rows
NCORES = 8
NS = N // NCORES         # 256 rows per core
FREE = NS * H            # 8192 columns per core
BLK = 512                # matmul free-dim tile (one fp32 PSUM bank)
NB = FREE // BLK         # 16 blocks

BF16 = mybir.dt.bfloat16
F32 = mybir.dt.float32
NPBF16 = jnp.bfloat16.dtype


def _build_qkv_module():
    nc = bacc.Bacc("TRN2", target_bir_lowering=False)
    xin = nc.dram_tensor("x_sh", [C, FREE], BF16, kind="ExternalInput")
    win = nc.dram_tensor("wT", [C, 2 * C], BF16, kind="ExternalInput")
    qout = nc.dram_tensor("qkv_sh", [2 * C, FREE], BF16, kind="ExternalOutput")

    with tile.TileContext(nc) as tc:
        with tc.tile_pool(name="wp", bufs=1) as wp, \
             tc.tile_pool(name="xp", bufs=8) as xp, \
             tc.tile_pool(name="pp", bufs=4, space="PSUM") as pp, \
             tc.tile_pool(name="op", bufs=4) as op:
            wts = []
            for kc in range(4):
                wt = wp.tile([128, 2 * C], BF16, tag=f"w{kc}")
                nc.sync.dma_start(wt[:], win[kc * 128:(kc + 1) * 128, :])
                wts.append(wt)
            for b in range(NB):
                xts = []
                for kc in range(4):
                    xt = xp.tile([128, BLK], BF16, tag="xt")
                    nc.sync.dma_start(
                        xt[:], xin[kc * 128:(kc + 1) * 128,
                                   b * BLK:(b + 1) * BLK])
                    xts.append(xt)
                for mc in range(8):
                    ps = pp.tile([128, BLK], F32, tag="ps")
                    for kc in range(4):
                        nc.tensor.matmul(
                            ps[:],
                            lhsT=wts[kc][:, mc * 128:(mc + 1) * 128],
                            rhs=xts[kc][:],
                            start=(kc == 0), stop=(kc == 3))
                    ot = op.tile([128, BLK], BF16, tag="ot")
                    nc.any.tensor_copy(ot[:], ps[:])
                    nc.sync.dma_start(
                        qout[mc * 128:(mc + 1) * 128, b * BLK:(b + 1) * BLK],
                        ot[:])
    nc.compile()
    return nc


# ---------------------------------------------------------------------------
# Jitted device pipeline (built once, cached).
# ---------------------------------------------------------------------------

_STATE = {}


def _get_state():
    if _STATE:
        return _STATE
    install_neuronx_cc_hook()
    nc = _build_qkv_module()
    partition_name = (nc.partition_id_tensor.name
                      if nc.partition_id_tensor else None)
    in_names, out_names, out_avals = [], [], []
    for alloc in nc.m.functions[0].allocations:
        if not isinstance(alloc, mybir.MemoryLocationSet):
            continue
        name = alloc.memorylocations[0].name
        if alloc.kind == "ExternalInput":
            if name != partition_name:
                in_names.append(name)
        elif alloc.kind == "ExternalOutput":
            out_names.append(name)
            out_avals.append(jax.core.ShapedArray(
                tuple(alloc.tensor_shape), mybir.dt.np(alloc.dtype)))
    all_in = list(in_names) + list(out_names)
    if partition_name is not None:
        all_in.append(partition_name)

    def bass_body(x_sh, wT, zeros):
        ops = [x_sh, wT, zeros]
        if partition_name is not None:
            ops.append(partition_id_tensor())
        outs = _bass_exec_p.bind(
            *ops, out_avals=tuple(out_avals), in_names=tuple(all_in),
            out_names=tuple(out_names), lowering_input_output_aliases=(),
            sim_require_finite=True, sim_require_nnan=True, nc=nc)
        return outs[0]

    def tail_body(qkv_sh, x_sh, all_emb, g1, g2, b1, b2):
        # qkv_sh [2C, FREE] bf16; x_sh [C, FREE] bf16 (both shard-local)
        qkv = qkv_sh.astype(jnp.float32).reshape(N_HEAD, 128, NS, H)
        q = qkv[:, :32]
        k = qkv[:, 32:64]
        v = qkv[:, 64:]
        q_emb = all_emb[:32]
        k_emb = all_emb[32:64]
        v_emb = all_emb[64:]
        qk = jnp.einsum('hcni,hcnj->hnij', q, k)
        qr = jnp.einsum('hcni,cij->hnij', q, q_emb)
        kr = jnp.einsum('hcni,cij->hnij', k, k_emb)
        sim = jax.nn.softmax(qk + qr + kr, axis=3)
        sv = jnp.einsum('hnij,hcnj->hcni', sim, v)       # (8,64,NS,H)
        sve = jnp.einsum('hnij,cij->hcni', sim, v_emb)
        cnt = float(N * H)
        s1 = jax.lax.psum(sv.sum(axis=(2, 3)), 'core')
        sq1 = jax.lax.psum((sv * sv).sum(axis=(2, 3)), 'core')
        s2 = jax.lax.psum(sve.sum(axis=(2, 3)), 'core')
        sq2 = jax.lax.psum((sve * sve).sum(axis=(2, 3)), 'core')
        m1 = s1 / cnt
        va1 = sq1 / cnt - m1 * m1
        m2 = s2 / cnt
        va2 = sq2 / cnt - m2 * m2
        a1 = g1 * jax.lax.rsqrt(va1 + BN_EPS)
        a2 = g2 * jax.lax.rsqrt(va2 + BN_EPS)
        shift = a1 * m1 + a2 * m2 - b1 - b2
        xr = x_sh.astype(jnp.float32).reshape(N_HEAD, 64, NS, H)
        out = (sv * a1[:, :, None, None] + sve * a2[:, :, None, None]
               - shift[:, :, None, None] + xr)
        out = jnp.maximum(out, 0.0)
        return out.reshape(C, FREE).astype(jnp.bfloat16)

    devices = jax.devices()[:NCORES]
    mesh = Mesh(np.asarray(devices), ("core",))
    PS = PartitionSpec
    shard = NamedSharding(mesh, PS("core"))
    repl = NamedSharding(mesh, PS())

    bass_fn = jax.jit(
        shard_map(bass_body, mesh=mesh,
                  in_specs=(PS("core"),) * 3, out_specs=PS("core"),
                  check_rep=False),
        donate_argnums=(2,), keep_unused=True)
    tail_fn = jax.jit(
        shard_map(tail_body, mesh=mesh,
                  in_specs=(PS("core"), PS("core"), PS(), PS(), PS(), PS(),
                            PS()),
                  out_specs=PS("core"), check_rep=False))

    _STATE.update(dict(bass_fn=bass_fn, tail_fn=tail_fn, mesh=mesh,
                       shard=shard, repl=repl,
                       out_shape=tuple(out_avals[0].shape),
                       out_dtype=out_avals[0].dtype))
    return _STATE


def _prep_host(x, w_qkv, relative, bn_gamma, bn_beta):
    """Host-side prep: permutes + dtype casts + constant tables."""
    # cast first (halves the bytes the permute moves), then permute
    xbf = x.astype(NPBF16)                                 # (B,C,H,W,T)
    # rows n=(b,t,w) shard s covers n in [s*NS,(s+1)*NS); per-core layout
    # (C, (n_local, h)): (B,C,H,W,T) -> (B,T,W,C,H) -> (cores, NS, C, H)
    xp_rows = np.ascontiguousarray(np.transpose(xbf, (0, 4, 3, 1, 2))
                                   ).reshape(NCORES, NS, C, H)
    xs_cat = np.ascontiguousarray(xp_rows.transpose(0, 2, 1, 3)
                                  ).reshape(NCORES * C, FREE)
    wT = np.ascontiguousarray(w_qkv.T).astype(NPBF16)
    wt_cat = np.concatenate([wT] * NCORES, axis=0)

    ar = np.arange(H)
    rel_idx = ar[:, None] - ar[None, :] + H - 1
    all_emb = np.ascontiguousarray(relative[:, rel_idx]).astype(np.float32)

    hh = np.arange(N_HEAD)[:, None]
    ch = np.arange(64)[None, :]
    g1 = bn_gamma[hh * 128 + 2 * ch].astype(np.float32)
    g2 = bn_gamma[hh * 128 + 2 * ch + 1].astype(np.float32)
    b1 = bn_beta[hh * 128 + 2 * ch].astype(np.float32)
    b2 = bn_beta[hh * 128 + 2 * ch + 1].astype(np.float32)
    return xs_cat, wt_cat, all_emb, g1, g2, b1, b2


def _run_device(xs_cat, wt_cat, all_emb, g1, g2, b1, b2):
    """Full device phase: H2D, bass qkv, XLA tail (with BN all-reduce), D2H.

    Returns out_cat (NCORES*C, FREE) bf16 numpy.
    """
    st = _get_state()
    zeros = jnp.zeros((NCORES * st["out_shape"][0],) + st["out_shape"][1:],
                      st["out_dtype"])
    x_dev = jax.device_put(xs_cat, st["shard"])
    qkv_cat = st["bass_fn"](x_dev, wt_cat, zeros)
    out_cat = st["tail_fn"](qkv_cat, x_dev, all_emb, g1, g2, b1, b2)
    return np.asarray(out_cat)


def _finish_host(out_cat, x):
    # permute in bf16 (half the bytes), upcast once at the end
    out = out_cat.reshape(NCORES, C, NS, H)
    out = out.transpose(0, 2, 1, 3).reshape(B, T, W, C, H)
    out = np.ascontiguousarray(out.transpose(0, 3, 4, 2, 1))  # (B,C,H,W,T)
    return out.astype(np.float32)


def kernel(x, w_qkv, relative, bn_gamma, bn_beta):
    x = np.asarray(x, dtype=np.float32)
    w_qkv = np.asarray(w_qkv, dtype=np.float32)
    relative = np.asarray(relative, dtype=np.float32)
    bn_gamma = np.asarray(bn_gamma, dtype=np.float32)
    bn_beta = np.asarray(bn_beta, dtype=np.float32)

    prepped = _prep_host(x, w_qkv, relative, bn_gamma, bn_beta)
    out_cat = _run_device(*prepped)
    return _finish_host(out_cat, x)
